# revision 31
# baseline (speedup 1.0000x reference)
"""Trainium2 Bass kernel: Based linear attention (poly feature map, causal, normalized).

Full inputs q,k,v: [1, 16, 4096, 16] fp32. Output: [1, 16, 4096, 16] fp32.
Sharding: 16 heads over 8 cores (2 heads/core); each head is independent.

Algorithm (per head): chunked quadratic-state linear attention, C=128.
  P = 1 + s + 0.5 s^2 with s = u.k, u = q/sqrt(D).
  Intra chunk: stp[j,i] = [1|k_j].[1|u_i] = 1+s ; P = Square(stp/sqrt2) masked
  (j<=i) plus 0.5-masked (trih matmul). Cross chunk, explicit quadratic
  features: q2[i,(p,r)] = 0.5 u_p u_r (PE-transposed to [f,i]),
  k2[j,(p,r)] = k_p k_r; states M2[f,d'] = sum k2^T v', M1[e,d'] = sum
  [1|k]^T v' accumulate in PSUM; numerator = intra matmuls + q2t^T @ M2 +
  [1|u]^T-read of M1, all accumulated in one PSUM tile (v' = [v|1] carries
  the normalizer z in channel 16). Normalization batched at the end.
  k/a transposed feature tiles come from DMA xbar transposes.
"""
import numpy as np
from contextlib import ExitStack

import concourse.bass as bass
import concourse.bacc as bacc
import concourse.tile as tile
import concourse.mybir as mybir
from bass_rust import add_dep_helper
from concourse.masks import make_upper_triangular
from concourse.bass_utils import run_bass_kernel_spmd

B, H, S, D = 1, 16, 4096, 16
NCORES = 8
HPC = H // NCORES  # heads per core (2)
C = 128            # chunk (positions)
NCH = S // C       # 32 chunks
D1 = D + 1         # 17
F2 = 256           # quadratic features (p,r)
FP = 32            # padded feature width for xbar transpose (NCH*FP % 128 == 0, FP % 16 == 0)
dt = mybir.dt.float32
bt = mybir.dt.bfloat16
SCALE = 1.0 / np.sqrt(D)
RT2I = 1.0 / np.sqrt(2.0)
Alu = mybir.AluOpType
Act = mybir.ActivationFunctionType


def _ap(base_ap, offset_ap, dims):
    """AP on the same tensor as `base_ap`: partition dim kept, free dims replaced."""
    return bass.AP(tensor=base_ap.tensor, offset=offset_ap.offset,
                   ap=[base_ap.ap[0]] + dims)


def _build_core(nc, pools, q_d, k_d, v_d, o_d):
    (ident, trih, mask, wq2), bulk, sb, sbb, snapp, ps_stp, ps_q2t, ps_num, ps_state, ps_kt = pools

    # ---- raw loads (both heads): [h, S, D] -> [128, h, NCH, D] ----
    qraw = bulk.tile([128, HPC, NCH, D], dt, tag="qraw")
    kraw = bulk.tile([128, HPC, NCH, D], dt, tag="kraw")
    vraw = bulk.tile([128, HPC, NCH, D], dt, tag="vraw")
    NQ = NCH // 2

    # loads/prep/xbar are interleaved below (per half)
    # ---- feature tensors ----
    # kb = [1|k] (17 used, padded to 20 for the DMA xbar transpose), ab = [1|u]
    kb = bulk.tile([128, HPC, NCH, FP], bt, tag="kb")
    ab = bulk.tile([128, HPC, NCH, FP], bt, tag="ab")
    vb = bulk.tile([128, HPC, NCH, D1], bt, tag="vb")
    warm = bulk.tile([1, 1], dt, tag="actwarm")
    nc.vector.memset(warm[:], 1.0)
    nc.scalar.activation(warm[:], warm[:], Act.Square)
    nc.vector.memset(kb[:, :, :, 0:1], 1.0)
    nc.vector.memset(ab[:, :, :, 0:1], 1.0)
    nc.vector.memset(vb[:, :, :, D : D + 1], 1.0)
    nc.vector.memset(kb[:, :, :, D1:FP], 0.0)
    nc.vector.memset(ab[:, :, :, D1:FP], 0.0)

    # real xbar layout: out[r, g, j] = in[j, 128*g + r]; with FP=32 each
    # 128-col group g packs 4 chunks at partition bases {0,32,64,96}.
    ktp = bulk.tile([128, HPC, NCH // 4, 128], bt, tag="ktp")
    atp = bulk.tile([128, HPC, NCH // 4, 128], bt, tag="atp")
    NQ4 = NCH // 2
    NG4 = NCH // 2 // 4  # xbar groups per load half

    def emit_load(qt):
        cs = slice(qt * NQ4, (qt + 1) * NQ4)
        nc.sync.dma_start(kraw[:, :, cs], k_d[:, :, cs])
        nc.sync.dma_start(qraw[:, :, cs], q_d[:, :, cs])

    def emit_vload(qt):
        cs = slice(qt * NQ4, (qt + 1) * NQ4)
        nc.sync.dma_start(vraw[:, :, cs], v_d[:, :, cs])

    def emit_prep(qt, c_lo=None, c_hi=None, g_lo=None, g_hi=None):
        c_lo = qt * NQ4 if c_lo is None else c_lo
        c_hi = (qt + 1) * NQ4 if c_hi is None else c_hi
        g_lo = c_lo // 4 if g_lo is None else g_lo
        g_hi = c_hi // 4 if g_hi is None else g_hi
        cs = slice(c_lo, c_hi)
        nc.scalar.copy(kb[:, :, cs, 1 : D + 1], kraw[:, :, cs])
        nc.scalar.mul(ab[:, :, cs, 1 : D + 1], qraw[:, :, cs], SCALE)
        if g_hi > g_lo:
            gs = slice(g_lo, g_hi)
            xcs = slice(g_lo * 4, g_hi * 4)
            for h in range(HPC):
                nc.sync.dma_start_transpose(ktp[:, h, gs], kb[:, h, xcs])
                nc.sync.dma_start_transpose(atp[:, h, gs], ab[:, h, xcs])

    def emit_pe_transp(g):
        # PE transposes for group g (4 chunks) into ktp/atp, 2 batched copies
        tp = ps_kt.tile([128, HPC, 2, 128], bt, tag="kt_ps")
        for h in range(HPC):
            kb_slab = _ap(kb[:], kb[:, h, 4 * g, 0:1], [[1, 128]])
            ab_slab = _ap(ab[:], ab[:, h, 4 * g, 0:1], [[1, 128]])
            nc.tensor.matmul(tp[:, h, 0, :], kb_slab, ident[:], start=True,
                             stop=True, skip_group_check=True, is_transpose=True)
            nc.tensor.matmul(tp[:, h, 1, :], ab_slab, ident[:], start=True,
                             stop=True, skip_group_check=True, is_transpose=True)
        gstr = (NCH // 4) * 128
        kdst = _ap(ktp[:], ktp[:, 0, g, 0:1], [[gstr, HPC], [1, 128]])
        ksrc = _ap(tp[:], tp[:], [[256, HPC], [1, 128]])
        nc.vector.tensor_copy(kdst, ksrc)
        adst = _ap(atp[:], atp[:, 0, g, 0:1], [[gstr, HPC], [1, 128]])
        asrc = _ap(tp[:], tp[:, 0, 1, 0:1], [[256, HPC], [1, 128]])
        nc.vector.tensor_copy(adst, asrc)

    def emit_vprep(qt):
        cs = slice(qt * NQ4, (qt + 1) * NQ4)
        nc.scalar.copy(vb[:, :, cs, 0:D], vraw[:, :, cs])

    # all loads issued first (independent, no SP blocking); chunks 0-7
    # prepped + PE-transposed (fast path); groups 2-3 via xbar; half-1
    # prep mid-loop.
    emit_load(0)
    emit_vload(0)
    emit_load(1)
    emit_vload(1)
    emit_prep(0, c_lo=0, c_hi=8, g_lo=0, g_hi=0)
    emit_pe_transp(0)
    emit_pe_transp(1)
    emit_vprep(0)
    emit_prep(0, c_lo=8, c_hi=16, g_lo=2, g_hi=4)

    o_sb = bulk.tile([128, HPC, NCH, D], dt, tag="osb")

    # persistent PSUM state: [128, h, 3, 17] = (M2a, M2b, M1)
    st = ps_state.tile([128, HPC, 3, D1], dt, tag="st")
    nc.vector.memset(st[:], 0.0)

    snap_prev = None
    prev_snap_op = None
    q2t_sb_prev = None
    pt_prev = None
    cdata = {}

    # software pipeline: at iteration c, issue front-end for chunk c
    # (builds, transposes, scores, square, mask, q2t copy) and back-end for
    # chunk c-1 (readout matmuls, state update, snapshot, tot copy).
    for c in range(NCH + 1):
        if c == 4:
            emit_prep(1)
        if c == 7:
            emit_vprep(1)
        if c < NCH:
            # --- quadratic feature builds, batched over 4 chunks (4D TT) ---
            if c % 4 == 0:
                q2b = sbb.tile([128, HPC, 4, 192], bt, tag="q2b")
                k2b = sbb.tile([128, HPC, 4, 192], bt, tag="k2b")
                for h in range(HPC):
                    a0 = _ap(ab[:], ab[:, h, c, 1:2], [[FP, 4], [0, D], [1, 8]])
                    a1 = _ap(ab[:], ab[:, h, c, 1:2], [[FP, 4], [1, D], [0, 8]])
                    nc.vector.tensor_mul(q2b[:, h, :, 0:128], a0, a1)
                    a2 = _ap(ab[:], ab[:, h, c, 9:10], [[FP, 4], [1, 8], [0, 8]])
                    a3 = _ap(ab[:], ab[:, h, c, 9:10], [[FP, 4], [0, 8], [1, 8]])
                    nc.vector.tensor_mul(q2b[:, h, :, 128:192], a2, a3)
                    kk0 = _ap(kb[:], kb[:, h, c, 1:2], [[FP, 4], [0, D], [1, 8]])
                    kk1 = _ap(kb[:], kb[:, h, c, 1:2], [[FP, 4], [1, D], [0, 8]])
                    nc.gpsimd.tensor_mul(k2b[:, h, :, 0:128], kk0, kk1)
                    kk2 = _ap(kb[:], kb[:, h, c, 9:10], [[FP, 4], [1, 8], [0, 8]])
                    kk3 = _ap(kb[:], kb[:, h, c, 9:10], [[FP, 4], [0, 8], [1, 8]])
                    nc.gpsimd.tensor_mul(k2b[:, h, :, 128:192], kk2, kk3)
                cur_q2b, cur_k2b = q2b, k2b
            q2 = cur_q2b[:, :, c % 4]
            k2 = cur_k2b[:, :, c % 4]

            # --- PE: intra scores (first: feeds Square -> mask chain) ---
            stp = ps_stp.tile([128, HPC, 128], dt, tag="stp")
            g, p0 = c // 4, 32 * (c % 4)
            for h in range(HPC):
                nc.tensor.matmul(stp[:, h, :], ktp[p0 : p0 + D1, h, g, :],
                                 atp[p0 : p0 + D1, h, g, :],
                                 start=True, stop=True, skip_group_check=True,
                                 tile_position=(p0, 0))

            # --- PE: q2 transposes -> bf16 PSUM (2-chunk shared tile) ---
            if c % 2 == 0:
                q2t_ps = ps_q2t.tile([128, 2, HPC, 2, 128], bt, tag="q2t")
                cur_q2t_ps = q2t_ps
            for h in range(HPC):
                nc.tensor.matmul(cur_q2t_ps[:, c % 2, h, 0, :],
                                 q2[:, h, 0:128], ident[:], start=True,
                                 stop=True, skip_group_check=True,
                                 is_transpose=True)
                nc.tensor.matmul(cur_q2t_ps[0:64, c % 2, h, 1, :],
                                 q2[:, h, 128:192], ident[:], start=True,
                                 stop=True, skip_group_check=True,
                                 is_transpose=True)

            # --- Act: Square; DVE: causal mask then q2t copy (x0.5) ---
            sq = sb.tile([128, HPC, 128], bt, tag="sq")
            nc.scalar.activation(sq[:], stp[:], Act.Square, scale=RT2I)
            pt = sb.tile([128, HPC, 128], bt, tag="pt")
            mask_bc = _ap(mask[:], mask[:], [[0, HPC], [1, 128]])
            nc.vector.tensor_mul(pt[:], sq[:], mask_bc)
            if c % 2 == 1:
                q2t_sb2 = sb.tile([128, 2, HPC, 2, 128], bt, tag="q2tsb")
                nc.vector.tensor_scalar_mul(q2t_sb2[:], cur_q2t_ps[:],
                                            wq2[:, 0:1])
                cdata[c - 1] = cdata[c - 1][:2] + (q2t_sb2[:, 0],) + cdata[c - 1][3:]
                q2t_sb = q2t_sb2[:, 1]
            else:
                q2t_sb = None
            cdata[c] = (q2, k2, q2t_sb, pt)

        b = c - 1
        if b >= 0:
            q2_b, k2_b, q2t_b, pt_b = cdata.pop(b)
            # --- PE: readout matmuls -> num PSUM (8-chunk group tile) ---
            if b % 8 == 0:
                num8 = ps_num.tile([128, 8, HPC, D1], dt, tag="num8")
                cur_num8 = num8
            num = cur_num8[:, b % 8]
            for h in range(HPC):
                mms = []
                mms.append(nc.tensor.matmul(num[:, h, :], pt_b[:, h, :],
                                            vb[:, h, b, :], start=True,
                                            stop=False))
                mms.append(nc.tensor.matmul(num[:, h, :], trih[:],
                                            vb[:, h, b, :], start=False,
                                            stop=(b == 0)))
                if b > 0:
                    mms.append(nc.tensor.matmul(num[:, h, :],
                                                q2t_b[:, h, 0, :],
                                                snap_prev[:, h, 0, :],
                                                start=False, stop=False))
                    mms.append(nc.tensor.matmul(num[:, h, :],
                                                q2t_b[0:64, h, 1, :],
                                                snap_prev[0:64, h, 1, :],
                                                start=False, stop=False))
                    gb, pb = b // 4, 32 * (b % 4)
                    mms.append(nc.tensor.matmul(num[:, h, :],
                                                atp[pb : pb + D1, h, gb, :],
                                                snap_prev[pb : pb + D1, h, 2, :],
                                                start=False, stop=True,
                                                tile_position=(pb, 0)))
                for m0, m1 in zip(mms, mms[1:]):
                    add_dep_helper(m1.ins, m0.ins, reason="num accum order")

            # --- PE: state update (after previous snapshot read) ---
            umms = []
            for h in range(HPC) if b < NCH - 1 else []:
                umms.append(nc.tensor.matmul(st[:, h, 0, :],
                                             k2_b[:, h, 0:128], vb[:, h, b, :],
                                             start=False, stop=False,
                                             skip_group_check=True))
                umms.append(nc.tensor.matmul(st[0:64, h, 1, :],
                                             k2_b[:, h, 128:192], vb[:, h, b, :],
                                             start=False, stop=False,
                                             skip_group_check=True))
                for rb in range(4):
                    umms.append(nc.tensor.matmul(st[32 * rb : 32 * rb + D1, h, 2, :],
                                                 kb[:, h, b, 0:D1], vb[:, h, b, :],
                                                 start=False, stop=False,
                                                 skip_group_check=True,
                                                 tile_position=(0, 32 * rb)))
            if prev_snap_op is not None:
                for m in umms:
                    add_dep_helper(m.ins, prev_snap_op.ins,
                                   reason="state WAR after snapshot")

            # --- Pool: snapshot state; tot copy ---
            if b < NCH - 1:
                snap = snapp.tile([128, HPC, 3, D1], bt, tag="snap")
                cp = nc.scalar.copy(snap[:], st[:])
                for m in umms:
                    add_dep_helper(cp.ins, m.ins, reason="snapshot after update")
                snap_prev = snap
                prev_snap_op = cp
            # --- normalize straight from PSUM every 8 chunks; store ---
            if b % 8 == 7:
                g0 = b - 7
                csb = slice(g0, b + 1)
                rec = bulk.tile([128, 8, HPC, 1], dt, tag=f"rec{b}")
                nc.vector.reciprocal(rec[:], cur_num8[:, :, :, D : D + 1])
                rec_bc = _ap(rec[:], rec[:], [[1, HPC], [HPC, 8], [0, D]])
                num_r = _ap(cur_num8[:], cur_num8[:],
                            [[D1, HPC], [HPC * D1, 8], [1, D]])
                nc.vector.tensor_mul(o_sb[:, :, csb], num_r, rec_bc)
                nc.sync.dma_start(o_d[:, :, csb], o_sb[:, :, csb])

    # ---- epilogue (emitted per half from the loop): nothing left here ----


def build_program():
    nc = bacc.Bacc("TRN2", target_bir_lowering=False, debug=False)
    q_d = nc.dram_tensor("q", [128, HPC, NCH, D], dt, kind="ExternalInput")
    k_d = nc.dram_tensor("k", [128, HPC, NCH, D], dt, kind="ExternalInput")
    v_d = nc.dram_tensor("v", [128, HPC, NCH, D], dt, kind="ExternalInput")
    o_d = nc.dram_tensor("out", [128, HPC, NCH, D], dt, kind="ExternalOutput")

    with tile.TileContext(nc) as tc, ExitStack() as ctx:
        constp = ctx.enter_context(tc.tile_pool(name="const", bufs=1))
        bulk = ctx.enter_context(tc.tile_pool(name="bulk", bufs=1))
        sb = ctx.enter_context(tc.tile_pool(name="sb", bufs=12))
        sbb = ctx.enter_context(tc.tile_pool(name="sbb", bufs=3))
        snapp = ctx.enter_context(tc.tile_pool(name="snap", bufs=3))
        ps_stp = ctx.enter_context(tc.tile_pool(name="ps_stp", bufs=2, space="PSUM"))
        ps_q2t = ctx.enter_context(tc.tile_pool(name="ps_q2t", bufs=2, space="PSUM"))
        ps_num = ctx.enter_context(tc.tile_pool(name="ps_num", bufs=2, space="PSUM"))
        ps_state = ctx.enter_context(tc.tile_pool(name="ps_st", bufs=1, space="PSUM"))
        ps_kt = ctx.enter_context(tc.tile_pool(name="ps_kt", bufs=1, space="PSUM"))

        from concourse.masks import make_identity
        ident = constp.tile([128, 128], bt)
        make_identity(nc, ident)
        trih = constp.tile([128, 128], bt)
        make_upper_triangular(nc, trih, val=0.5, diag=True)
        wq2 = constp.tile([128, 1], dt)
        nc.vector.memset(wq2[0:64], 0.5)
        nc.vector.memset(wq2[64:128], 1.0)
        mask = constp.tile([128, 128], bt)
        make_upper_triangular(nc, mask, val=1.0, diag=True)

        pools = ((ident, trih, mask, wq2), bulk, sb, sbb, snapp, ps_stp, ps_q2t, ps_num, ps_state, ps_kt)
        _build_core(nc, pools, q_d, k_d, v_d, o_d)

    nc.compile()
    return nc


_NC = None


def _perm_in(x):
    x = x.reshape(HPC, NCH, 128, D)
    return np.ascontiguousarray(np.transpose(x, (2, 0, 1, 3)))


def kernel(q: np.ndarray, k: np.ndarray, v: np.ndarray) -> np.ndarray:
    global _NC
    if _NC is None:
        _NC = build_program()
    q = np.asarray(q, dtype=np.float32).reshape(H, S, D)
    k = np.asarray(k, dtype=np.float32).reshape(H, S, D)
    v = np.asarray(v, dtype=np.float32).reshape(H, S, D)
    in_maps = []
    for i in range(NCORES):
        sl = slice(i * HPC, (i + 1) * HPC)
        in_maps.append({
            "q": _perm_in(q[sl]),
            "k": _perm_in(k[sl]),
            "v": _perm_in(v[sl]),
        })
    res = run_bass_kernel_spmd(_NC, in_maps, core_ids=list(range(NCORES)))
    outs = []
    for i in range(NCORES):
        o = res.results[i]["out"]
        outs.append(np.transpose(o, (1, 2, 0, 3)).reshape(HPC, S, D))
    return np.concatenate(outs, axis=0).reshape(B, H, S, D)

